# revision 3
# baseline (speedup 1.0000x reference)
"""Trainium2 Bass kernel for nn_Conv2DLayer_16011638080159 — fp8e3m4 col-tiled
PE-matvec, v12 (v8 + split accumulation chains -> fill-limited waves; ACT copy + DVE add combine).

Math: out = C * (x @ weight.sum(0))   with x [524288, 512], weight [9, 512].

v7 (124.6 us): PE busy only 62.9 us (column tiling works); bound by the DMA
stream (~335 GB/s) and a slow strided SWDGE output flush (~8 us tail). v8:
  - Host pre-packs x per core into the exact per-tile stream layout
    [tile, partition, chunk, quad-block, f]: every DMA half-tile is one
    contiguous 8 KB run per partition (max ring efficiency).
  - Column-group q of the PE now owns output rows [q*16384, (q+1)*16384):
    the staging quadrant row is contiguous in DRAM, so the 4 final flushes
    are plain 64 KB copies, 2 on each HWDGE ring.
  - NF=4096 tiles (2 MB, 2 quads per tile), 6-deep buffering.

Structure per tile t (2 quads kk): 32 interleaved matmuls
(tile_position=(0,32q)) accumulate quad (t,kk) into one PSUM bank at
partitions {0,32,64,96}; one [128,512] drain per quad (ScalarE/VectorE
alternating) into staging column k = 2t+kk.
"""

import numpy as np
import ml_dtypes

import concourse.bacc as bacc
import concourse.bass as bass
import concourse.tile as tile
from concourse import mybir
from concourse.bass_utils import run_bass_kernel_spmd

BF16 = ml_dtypes.bfloat16
E3M4 = ml_dtypes.float8_e3m4

B = 524288         # total rows
C = 512            # row length (contraction)
N_CORES = 8
BS = B // N_CORES  # 65536 rows per core
P = 128            # SBUF partitions / PE contraction per matmul
NCHUNK = C // P    # 4 c-chunks
NF = 4096          # x columns (= output rows) per DMA tile (2 quads)
NT = BS // NF      # 16 tiles per core
NK = BS // 512 // 4  # 32 column-blocks per quadrant
FPB = NCHUNK * NF  # 16384 free bytes (elems) per partition per tile

_NC_CACHE = None
LAST_RESULT = None


def _build() -> bass.Bass:
    nc = bacc.Bacc(None, target_bir_lowering=False, debug=False)
    xt = nc.dram_tensor("xt", [NT * P, FPB], mybir.dt.float8e3, kind="ExternalInput")
    w = nc.dram_tensor("w", [P, NCHUNK], mybir.dt.bfloat16, kind="ExternalInput")
    out = nc.dram_tensor("out", [BS], mybir.dt.float32, kind="ExternalOutput")

    xv = xt.rearrange("(t p) u -> t p u", t=NT, p=P)
    ov = out.rearrange("(q m) -> q m", q=4)  # quadrant rows contiguous

    with tile.TileContext(nc) as tc:
        with (
            tc.tile_pool(name="const", bufs=1) as cpool,
            tc.tile_pool(name="xs", bufs=6) as xs,
            tc.psum_pool(name="ps", bufs=7) as ps,
            tc.tile_pool(name="res", bufs=1) as res,
        ):
            w_t = cpool.tile([P, NCHUNK], mybir.dt.bfloat16)
            nc.sync.dma_start(out=w_t[:], in_=w[:, :])
            o_t = res.tile([128, NK * 512], mybir.dt.float32)
            for t in range(NT):
                x_t = xs.tile([P, FPB], mybir.dt.float8e3)
                nc.sync.dma_start(out=x_t[:, 0:FPB // 2], in_=xv[t][:, 0:FPB // 2])
                nc.scalar.dma_start(out=x_t[:, FPB // 2:], in_=xv[t][:, FPB // 2:])
                for kk in range(2):
                    p_a = ps.tile([128, 512], mybir.dt.float32, tag="pq")
                    p_b = ps.tile([128, 512], mybir.dt.float32, tag="pq")
                    # two 2-deep accumulation chains per col-group, issue
                    # order j0(A) j2(B) j1(A) j3(B): every wave is
                    # fill-limited (alternation covers the retire latency)
                    for w, (pt, j, st, sp) in enumerate((
                        (p_a, 0, True, False), (p_b, 2, True, False),
                        (p_a, 1, False, True), (p_b, 3, False, True),
                    )):
                        for q in range(4):
                            off = j * NF + kk * 2048 + q * 512
                            nc.tensor.matmul(
                                pt[32 * q:32 * q + 1, :],
                                lhsT=w_t[:, j:j + 1],
                                rhs=x_t[:, off:off + 512],
                                start=st,
                                stop=sp,
                                tile_position=(0, 32 * q),
                            )
                    k = 2 * t + kk
                    dst = o_t[:, k * 512:(k + 1) * 512]
                    nc.scalar.copy(out=dst, in_=p_a[:])
                    nc.vector.tensor_add(dst, dst, p_b[:])
            for q in range(4):
                ring = nc.sync if q % 2 == 0 else nc.scalar
                ring.dma_start(out=ov[q:q + 1, :], in_=o_t[32 * q:32 * q + 1, :])
    nc.finalize()
    return nc


def _pack(xb_core: np.ndarray) -> np.ndarray:
    """[65536, 512] fp8 row-major -> [NT*P, FPB] per-tile stream layout.

    Xp[t, p, j, kk, q, f] = xT[j*128+p, q*16384 + (2t+kk)*512 + f]
    """
    xt_c = xb_core.T                                   # [512, 65536]
    v = xt_c.reshape(NCHUNK, P, 4, NT, 2, 512)         # j p q t kk f
    v = v.transpose(3, 1, 0, 4, 2, 5)                  # t p j kk q f
    return np.ascontiguousarray(v).reshape(NT * P, FPB)


def kernel(x: np.ndarray, weight: np.ndarray) -> np.ndarray:
    global _NC_CACHE, LAST_RESULT
    x = np.asarray(x, dtype=np.float32)
    weight = np.asarray(weight, dtype=np.float32)

    w_eff = (C * weight.sum(axis=0)).astype(np.float32)        # [C]
    w_sb = np.ascontiguousarray(
        w_eff.reshape(NCHUNK, P).T.astype(BF16))               # [P, NCHUNK]

    if _NC_CACHE is None:
        _NC_CACHE = _build()

    xb = x.astype(E3M4)
    in_maps = [
        {"xt": _pack(xb[i * BS:(i + 1) * BS]), "w": w_sb}
        for i in range(N_CORES)
    ]
    LAST_RESULT = run_bass_kernel_spmd(
        _NC_CACHE, in_maps, core_ids=list(range(N_CORES))
    )
    return np.concatenate([r["out"] for r in LAST_RESULT.results])
